# revision 15
# baseline (speedup 1.0000x reference)
"""BinaryLinear kernel for 8 Trainium2 NeuronCores.

Computes out = x @ sign(W).T + bias for x [8, 2048, 4096], W [4096, 4096],
bias [4096], all float32.

Strategy: data-parallel over the batch dim — core b handles x[b] ([2048
tokens, 4096 in]) with the full (binarized) weight matrix.

Per-core device kernel (Tile framework):
  - x[b].T is uploaded as bf16 [in=4096, tokens=2048] and kept SBUF-resident
    (16 MB).
  - sign(W).T is uploaded as bf16 [in=4096, out=4096] (+-1 is exact in bf16)
    and streamed column-block by column-block (one 128-wide out-feature block
    per iteration, double buffered).
  - TensorE computes out.T tiles: psum[o_tile 128, t 512] += wT_blk[k 128,
    o 128].T @ xT[k 128, t 512], accumulated over 32 k-tiles.
  - ScalarE evicts PSUM -> SBUF adding the bias (per-partition AP bias).
  - Output is written as out.T [4096, 2048] f32; host transposes back.

bf16 matmul runs at 1 cycle/row on the PE (fp32 needs 4), and rounding x to
bf16 against exact +-1 weights keeps relative error ~1e-3.
"""

import numpy as np
import ml_dtypes

B = 8
T = 2048
IN_F = 4096
OUT_F = 4096
N_CORES = 8
P = 128
KT = IN_F // P  # 32 contraction tiles
OT = OUT_F // P  # 32 out-feature tiles
TN = 512  # moving-operand free dim (one PSUM bank of f32; ISA caps mm num_elements at 512)
TT = T // TN  # 4 token slices

_compiled_nc = None


def build_program():
    import concourse.mybir as mybir
    import concourse.tile as tile
    from concourse import bacc

    nc = bacc.Bacc("TRN2", target_bir_lowering=False, debug=False)

    xT = nc.dram_tensor("xT", [IN_F, T], mybir.dt.bfloat16, kind="ExternalInput")
    # W pre-packed on host: wP[ot, p, kt*128 + o] = sign(W)[ot*128+o, kt*128+p]
    # so each per-ot block DMA is 8KB-contiguous per partition row.
    wP = nc.dram_tensor(
        "wP", [OT, P, KT, P], mybir.dt.bfloat16, kind="ExternalInput"
    )
    bv = nc.dram_tensor("biasv", [P, OT], mybir.dt.float32, kind="ExternalInput")
    oT = nc.dram_tensor("outT", [OUT_F, T], mybir.dt.float32, kind="ExternalOutput")

    xT_r = xT.ap().rearrange("(kt p) t -> p kt t", p=P)  # [128, 32, 2048]
    oT_r = oT.ap().rearrange("(ot p) t -> p ot t", p=P)  # [128, 32, 2048]

    CHUNK = 8  # k-tiles per phase-1 weight chunk
    NCH = KT // CHUNK  # 4 chunks per block

    def evict(nc, mybir, opool, oT_r, b_sb, psum, ot, tt):
        o_sb = opool.tile([P, TN], mybir.dt.float32, name=f"o_{ot}_{tt}", tag="o")
        nc.scalar.activation(
            o_sb[:],
            psum[:],
            mybir.ActivationFunctionType.Identity,
            bias=b_sb[:, ot : ot + 1],
        )
        nc.sync.dma_start(oT_r[:, ot, tt * TN : (tt + 1) * TN], o_sb[:])

    with tile.TileContext(nc) as tc:
        with (
            tc.tile_pool(name="xpool", bufs=KT) as xpool,
            tc.tile_pool(name="wcpool", bufs=2 * NCH) as wcpool,
            tc.tile_pool(name="wpool", bufs=3) as wpool,
            tc.tile_pool(name="bpool", bufs=1) as bpool,
            tc.tile_pool(name="opool", bufs=6) as opool,
            tc.tile_pool(name="pspool", bufs=8 * 512 // TN, space="PSUM") as pspool,
        ):
            wP_r = wP.ap().rearrange("ot p (c k) o -> ot p c k o", c=NCH)

            # Phase 1: the first TWO output blocks share one k-loop (8 PSUM
            # banks) so the PE consumes each arriving x tile twice — this
            # makes the x-streaming phase PE-bound instead of DMA-bound.
            # Their weights arrive in chunks interleaved with the x stream.
            wc = {}  # (ot, c) -> tile
            x_tiles = []

            def load_chunk(o2, c):
                w_t = wcpool.tile(
                    [P, CHUNK, P], mybir.dt.bfloat16, name=f"wc_{o2}_{c}", tag="wc"
                )
                nc.sync.dma_start(w_t[:], wP_r[o2, :, c])
                wc[(o2, c)] = w_t

            def load_chunk_pair(c):
                for o2 in range(2):
                    load_chunk(o2, c)

            def load_x(kt):
                x_t = xpool.tile([P, T], mybir.dt.bfloat16, name=f"x_{kt}", tag="x")
                nc.sync.dma_start(x_t[:], xT_r[:, kt, :])
                x_tiles.append(x_t)

            # Interleave chunk-pair and x-tile DMAs in consumption order.
            # The very first matmul needs only wc(0,0) + x0, so issue those
            # two first.
            load_chunk(0, 0)
            load_x(0)
            load_chunk(1, 0)
            for kt in range(1, 6):
                load_x(kt)
            # Bias is tiny but descriptor-heavy; keep it off the critical
            # startup path (first needed at the first eviction, ~60us in).
            b_sb = bpool.tile([P, OT], mybir.dt.float32, name="b_sb")
            nc.sync.dma_start(b_sb[:], bv.ap())
            load_chunk_pair(1)
            for kt in range(6, 14):
                load_x(kt)
            load_chunk_pair(2)
            for kt in range(14, 22):
                load_x(kt)
            load_chunk_pair(3)
            for kt in range(22, KT):
                load_x(kt)

            psums1 = [
                [
                    pspool.tile([P, TN], mybir.dt.float32, name=f"ps_{o2}_{tt}", tag="ps")
                    for tt in range(TT)
                ]
                for o2 in range(2)
            ]
            for kt in range(KT):
                for o2 in range(2):
                    lhsT = wc[(o2, kt // CHUNK)][:, kt % CHUNK, :]
                    for tt in range(TT):
                        nc.tensor.matmul(
                            psums1[o2][tt][:],
                            lhsT,
                            x_tiles[kt][:, tt * TN : (tt + 1) * TN],
                            start=(kt == 0),
                            stop=(kt == KT - 1),
                        )
            for o2 in range(2):
                for tt in range(TT):
                    evict(nc, mybir, opool, oT_r, b_sb, psums1[o2][tt], o2, tt)

            # Phase 2: remaining blocks against the resident x.
            for ot in range(2, OT):
                w_sb = wpool.tile(
                    [P, KT, P], mybir.dt.bfloat16, name=f"w_{ot}", tag="w"
                )
                nc.sync.dma_start(w_sb[:], wP.ap()[ot])

                # tt-outer: each PSUM bank finishes its 32-matmul group in a
                # burst and evicts while the next bank accumulates, so
                # evictions never pile up after the block's last matmul.
                for tt in range(TT):
                    psum = pspool.tile(
                        [P, TN], mybir.dt.float32, name=f"ps_{ot}_{tt}", tag="ps"
                    )
                    for kt in range(KT):
                        nc.tensor.matmul(
                            psum[:],
                            w_sb[:, kt, :],
                            x_tiles[kt][:, tt * TN : (tt + 1) * TN],
                            start=(kt == 0),
                            stop=(kt == KT - 1),
                        )
                    evict(nc, mybir, opool, oT_r, b_sb, psum, ot, tt)

    nc.compile()
    return nc


def prepare_inputs(x, weight, bias):
    """Host-side layout prep: transpose + cast per-core shards."""
    bf16 = ml_dtypes.bfloat16
    w_bin = np.where(weight >= 0, np.float32(1.0), np.float32(-1.0))
    # wP[ot, p, kt, o] = sign(W)[ot*128+o, kt*128+p] — per-ot weight blocks,
    # contiguous along (kt, o) so block DMAs are 8KB-contiguous per partition.
    wP_np = np.ascontiguousarray(
        w_bin.reshape(OT, P, KT, P).transpose(0, 3, 2, 1)
    ).astype(bf16)
    bv_np = np.ascontiguousarray(
        np.asarray(bias, dtype=np.float32).reshape(OT, P).T
    )  # [P, OT]; bias[o] at [o % 128, o // 128]
    in_maps = []
    for b in range(B):
        xT_np = np.ascontiguousarray(x[b].T).astype(bf16)  # [in, tokens]
        in_maps.append({"xT": xT_np, "wP": wP_np, "biasv": bv_np})
    return in_maps


def run(in_maps, trace=False, **kwargs):
    global _compiled_nc
    if _compiled_nc is None:
        _compiled_nc = build_program()
    from concourse.bass_utils import run_bass_kernel_spmd

    return run_bass_kernel_spmd(
        _compiled_nc, in_maps, list(range(N_CORES)), trace=trace, **kwargs
    )


def kernel(x, weight, bias):
    res = run(prepare_inputs(x, weight, bias))
    out = np.empty((B, T, OUT_F), dtype=np.float32)
    for b in range(B):
        out[b] = res.results[b]["outT"].T
    return out


# revision 17
# speedup vs baseline: 1.0263x; 1.0263x over previous
"""BinaryLinear kernel for 8 Trainium2 NeuronCores.

Computes out = x @ sign(W).T + bias for x [8, 2048, 4096], W [4096, 4096],
bias [4096], all float32.

Strategy: data-parallel over the batch dim — core b handles x[b] ([2048
tokens, 4096 in]) with the full (binarized) weight matrix.

Per-core device kernel (Tile framework):
  - x[b].T is uploaded as bf16 [in=4096, tokens=2048] and kept SBUF-resident
    (16 MB).
  - sign(W).T is uploaded as bf16 [in=4096, out=4096] (+-1 is exact in bf16)
    and streamed column-block by column-block (one 128-wide out-feature block
    per iteration, double buffered).
  - TensorE computes out.T tiles: psum[o_tile 128, t 512] += wT_blk[k 128,
    o 128].T @ xT[k 128, t 512], accumulated over 32 k-tiles.
  - ScalarE evicts PSUM -> SBUF adding the bias (per-partition AP bias).
  - Output is written as out.T [4096, 2048] f32; host transposes back.

bf16 matmul runs at 1 cycle/row on the PE (fp32 needs 4), and rounding x to
bf16 against exact +-1 weights keeps relative error ~1e-3.
"""

import numpy as np
import ml_dtypes

B = 8
T = 2048
IN_F = 4096
OUT_F = 4096
N_CORES = 8
P = 128
KT = IN_F // P  # 32 contraction tiles
OT = OUT_F // P  # 32 out-feature tiles
TN = 512  # moving-operand free dim (one PSUM bank of f32; ISA caps mm num_elements at 512)
TT = T // TN  # 4 token slices

_compiled_nc = None


def build_program():
    import concourse.mybir as mybir
    import concourse.tile as tile
    from concourse import bacc

    nc = bacc.Bacc("TRN2", target_bir_lowering=False, debug=False)

    xT = nc.dram_tensor("xT", [IN_F, T], mybir.dt.bfloat16, kind="ExternalInput")
    # W pre-packed on host: wP[ot, p, kt*128 + o] = sign(W)[ot*128+o, kt*128+p]
    # so each per-ot block DMA is 8KB-contiguous per partition row.
    wP = nc.dram_tensor(
        "wP", [OT, P, KT, P], mybir.dt.bfloat16, kind="ExternalInput"
    )
    bv = nc.dram_tensor("biasv", [P, OT], mybir.dt.float32, kind="ExternalInput")
    oT = nc.dram_tensor("outT", [OUT_F, T], mybir.dt.float32, kind="ExternalOutput")

    xT_r = xT.ap().rearrange("(kt p) t -> p kt t", p=P)  # [128, 32, 2048]
    oT_r = oT.ap().rearrange("(ot p) t -> p ot t", p=P)  # [128, 32, 2048]

    CHUNK = 8  # k-tiles per phase-1 weight chunk
    NCH = KT // CHUNK  # 4 chunks per block

    def evict(nc, mybir, opool, oT_r, b_sb, psum, ot, tt):
        o_sb = opool.tile([P, TN], mybir.dt.float32, name=f"o_{ot}_{tt}", tag="o")
        nc.scalar.activation(
            o_sb[:],
            psum[:],
            mybir.ActivationFunctionType.Identity,
            bias=b_sb[:, ot : ot + 1],
        )
        nc.sync.dma_start(oT_r[:, ot, tt * TN : (tt + 1) * TN], o_sb[:])

    with tile.TileContext(nc) as tc:
        with (
            tc.tile_pool(name="xpool", bufs=KT) as xpool,
            tc.tile_pool(name="wcpool", bufs=2 * NCH) as wcpool,
            tc.tile_pool(name="wpool", bufs=3) as wpool,
            tc.tile_pool(name="bpool", bufs=1) as bpool,
            tc.tile_pool(name="opool", bufs=6) as opool,
            tc.tile_pool(name="pspool", bufs=8 * 512 // TN, space="PSUM") as pspool,
        ):
            wP_r = wP.ap().rearrange("ot p (c k) o -> ot p c k o", c=NCH)

            # Phase 1: the first TWO output blocks share one k-loop (8 PSUM
            # banks) so the PE consumes each arriving x tile twice — this
            # makes the x-streaming phase PE-bound instead of DMA-bound.
            # Their weights arrive in chunks interleaved with the x stream.
            wc = {}  # (ot, c) -> tile
            x_tiles = []

            def load_chunk(o2, c):
                # Weight chunks ride SWDGE (gpsimd) so they engage the other
                # 8 DMA engines, in parallel with the x stream on HWDGE.
                w_t = wcpool.tile(
                    [P, CHUNK, P], mybir.dt.bfloat16, name=f"wc_{o2}_{c}", tag="wc"
                )
                nc.gpsimd.dma_start(w_t[:], wP_r[o2, :, c])
                wc[(o2, c)] = w_t

            def load_chunk_pair(c):
                for o2 in range(2):
                    load_chunk(o2, c)

            def load_x(kt):
                x_t = xpool.tile([P, T], mybir.dt.bfloat16, name=f"x_{kt}", tag="x")
                nc.sync.dma_start(x_t[:], xT_r[:, kt, :])
                x_tiles.append(x_t)

            # Interleave chunk-pair and x-tile DMAs in consumption order.
            # The very first matmul needs only wc(0,0) + x0, so issue those
            # two first.
            load_chunk(0, 0)
            load_x(0)
            load_chunk(1, 0)
            for kt in range(1, 6):
                load_x(kt)
            # Bias is tiny but descriptor-heavy; keep it off the critical
            # startup path (first needed at the first eviction, ~60us in).
            b_sb = bpool.tile([P, OT], mybir.dt.float32, name="b_sb")
            nc.sync.dma_start(b_sb[:], bv.ap())
            load_chunk_pair(1)
            for kt in range(6, 14):
                load_x(kt)
            load_chunk_pair(2)
            for kt in range(14, 22):
                load_x(kt)
            load_chunk_pair(3)
            for kt in range(22, KT):
                load_x(kt)

            psums1 = [
                [
                    pspool.tile([P, TN], mybir.dt.float32, name=f"ps_{o2}_{tt}", tag="ps")
                    for tt in range(TT)
                ]
                for o2 in range(2)
            ]
            for kt in range(KT):
                for o2 in range(2):
                    lhsT = wc[(o2, kt // CHUNK)][:, kt % CHUNK, :]
                    for tt in range(TT):
                        nc.tensor.matmul(
                            psums1[o2][tt][:],
                            lhsT,
                            x_tiles[kt][:, tt * TN : (tt + 1) * TN],
                            start=(kt == 0),
                            stop=(kt == KT - 1),
                        )
            for o2 in range(2):
                for tt in range(TT):
                    evict(nc, mybir, opool, oT_r, b_sb, psums1[o2][tt], o2, tt)

            # Phase 2: remaining blocks against the resident x.
            for ot in range(2, OT):
                w_sb = wpool.tile(
                    [P, KT, P], mybir.dt.bfloat16, name=f"w_{ot}", tag="w"
                )
                nc.sync.dma_start(w_sb[:], wP.ap()[ot])

                # tt-outer: each PSUM bank finishes its 32-matmul group in a
                # burst and evicts while the next bank accumulates, so
                # evictions never pile up after the block's last matmul.
                for tt in range(TT):
                    psum = pspool.tile(
                        [P, TN], mybir.dt.float32, name=f"ps_{ot}_{tt}", tag="ps"
                    )
                    for kt in range(KT):
                        nc.tensor.matmul(
                            psum[:],
                            w_sb[:, kt, :],
                            x_tiles[kt][:, tt * TN : (tt + 1) * TN],
                            start=(kt == 0),
                            stop=(kt == KT - 1),
                        )
                    evict(nc, mybir, opool, oT_r, b_sb, psum, ot, tt)

    nc.compile()
    return nc


def prepare_inputs(x, weight, bias):
    """Host-side layout prep: transpose + cast per-core shards."""
    bf16 = ml_dtypes.bfloat16
    x = np.asarray(x, dtype=np.float32)
    weight = np.asarray(weight, dtype=np.float32)
    bias = np.asarray(bias, dtype=np.float32)
    w_bin = np.where(weight >= 0, np.float32(1.0), np.float32(-1.0))
    # wP[ot, p, kt, o] = sign(W)[ot*128+o, kt*128+p] — per-ot weight blocks,
    # contiguous along (kt, o) so block DMAs are 8KB-contiguous per partition.
    wP_np = np.ascontiguousarray(
        w_bin.reshape(OT, P, KT, P).transpose(0, 3, 2, 1)
    ).astype(bf16)
    bv_np = np.ascontiguousarray(
        np.asarray(bias, dtype=np.float32).reshape(OT, P).T
    )  # [P, OT]; bias[o] at [o % 128, o // 128]
    in_maps = []
    for b in range(B):
        xT_np = np.ascontiguousarray(x[b].T).astype(bf16)  # [in, tokens]
        in_maps.append({"xT": xT_np, "wP": wP_np, "biasv": bv_np})
    return in_maps


def run(in_maps, trace=False, **kwargs):
    global _compiled_nc
    if _compiled_nc is None:
        _compiled_nc = build_program()
    from concourse.bass_utils import run_bass_kernel_spmd

    return run_bass_kernel_spmd(
        _compiled_nc, in_maps, list(range(N_CORES)), trace=trace, **kwargs
    )


def kernel(x, weight, bias):
    res = run(prepare_inputs(x, weight, bias))
    out = np.empty((B, T, OUT_F), dtype=np.float32)
    for b in range(B):
        out[b] = res.results[b]["outT"].T
    return out


# revision 20
# speedup vs baseline: 1.0292x; 1.0028x over previous
"""BinaryLinear kernel for 8 Trainium2 NeuronCores.

Computes out = x @ sign(W).T + bias for x [8, 2048, 4096], W [4096, 4096],
bias [4096], all float32.

Strategy: data-parallel over the batch dim — core b handles x[b] ([2048
tokens, 4096 in]) with the full (binarized) weight matrix.

Per-core device kernel (Tile framework):
  - x[b].T is uploaded as bf16 [in=4096, tokens=2048] and kept SBUF-resident
    (16 MB), one tile per 128-row k-slice so compute starts as slices land.
  - sign(W).T is uploaded as bf16 (+-1 is exact in bf16), host-packed into
    per-out-block tiles so every weight DMA is 8KB-contiguous per partition,
    and streamed one 128-wide out-feature block per iteration.
  - TensorE computes out.T tiles: psum[o_tile 128, t 512] += wT_blk[k 128,
    o 128].T @ xT[k 128, t 512], accumulated over 32 k-tiles.
  - Phase 1 interleaves the first TWO out-blocks in one k-loop (all 8 PSUM
    banks) so the x-streaming prologue is PE-bound, not DMA-bound; phase 2
    runs the remaining 30 blocks tt-outer against the resident x.
  - ScalarE evicts PSUM -> SBUF adding the bias (per-partition AP bias).
  - Output is written as out.T [4096, 2048] f32; host transposes back.

bf16 matmul runs at 1 cycle/row on the PE (fp32 needs 4), and rounding x to
bf16 against exact +-1 weights keeps relative error ~1.7e-3. Measured
~906-909us HW exec vs an 874us pure-matmul floor (4096 MMs x ~215ns): the
PE runs gap-free; overhead is the fixed engine preamble (~7us), the
critical-prefix DMA before the first matmul (~7us), and the Tile exit
drain (~12us).
"""

import numpy as np
import ml_dtypes

B = 8
T = 2048
IN_F = 4096
OUT_F = 4096
N_CORES = 8
P = 128
KT = IN_F // P  # 32 contraction tiles
OT = OUT_F // P  # 32 out-feature tiles
TN = 512  # moving-operand free dim (one PSUM bank of f32; ISA caps mm num_elements at 512)
TT = T // TN  # 4 token slices

_compiled_nc = None


def build_program():
    import concourse.mybir as mybir
    import concourse.tile as tile
    from concourse import bacc

    nc = bacc.Bacc("TRN2", target_bir_lowering=False, debug=False)

    xT = nc.dram_tensor("xT", [IN_F, T], mybir.dt.bfloat16, kind="ExternalInput")
    # W pre-packed on host: wP[ot, p, kt*128 + o] = sign(W)[ot*128+o, kt*128+p]
    # so each per-ot block DMA is 8KB-contiguous per partition row.
    wP = nc.dram_tensor(
        "wP", [OT, P, KT, P], mybir.dt.bfloat16, kind="ExternalInput"
    )
    bv = nc.dram_tensor("biasv", [P, OT], mybir.dt.float32, kind="ExternalInput")
    oT = nc.dram_tensor("outT", [OUT_F, T], mybir.dt.float32, kind="ExternalOutput")

    xT_r = xT.ap().rearrange("(kt p) t -> p kt t", p=P)  # [128, 32, 2048]
    oT_r = oT.ap().rearrange("(ot p) t -> p ot t", p=P)  # [128, 32, 2048]

    CHUNK = 8  # k-tiles per phase-1 weight chunk
    NCH = KT // CHUNK  # 4 chunks per block

    def evict(nc, mybir, opool, oT_r, b_sb, psum, ot, tt):
        o_sb = opool.tile([P, TN], mybir.dt.float32, name=f"o_{ot}_{tt}", tag="o")
        nc.scalar.activation(
            o_sb[:],
            psum[:],
            mybir.ActivationFunctionType.Identity,
            bias=b_sb[:, ot : ot + 1],
        )
        nc.sync.dma_start(oT_r[:, ot, tt * TN : (tt + 1) * TN], o_sb[:])

    with tile.TileContext(nc) as tc:
        with (
            tc.tile_pool(name="xpool", bufs=KT) as xpool,
            tc.tile_pool(name="wcpool", bufs=2 * NCH) as wcpool,
            tc.tile_pool(name="wpool", bufs=3) as wpool,
            tc.tile_pool(name="bpool", bufs=1) as bpool,
            tc.tile_pool(name="opool", bufs=6) as opool,
            tc.tile_pool(name="pspool", bufs=8 * 512 // TN, space="PSUM") as pspool,
        ):
            wP_r = wP.ap().rearrange("ot p (c k) o -> ot p c k o", c=NCH)

            # Phase 1: the first TWO output blocks share one k-loop (8 PSUM
            # banks) so the PE consumes each arriving x tile twice — this
            # makes the x-streaming phase PE-bound instead of DMA-bound.
            # Their weights arrive in chunks interleaved with the x stream.
            wc = {}  # (ot, c) -> tile
            x_tiles = []

            def load_chunk(o2, c):
                w_t = wcpool.tile(
                    [P, CHUNK, P], mybir.dt.bfloat16, name=f"wc_{o2}_{c}", tag="wc"
                )
                nc.sync.dma_start(w_t[:], wP_r[o2, :, c])
                wc[(o2, c)] = w_t

            def load_chunk_pair(c):
                for o2 in range(2):
                    load_chunk(o2, c)

            def load_x(kt):
                x_t = xpool.tile([P, T], mybir.dt.bfloat16, name=f"x_{kt}", tag="x")
                nc.sync.dma_start(x_t[:], xT_r[:, kt, :])
                x_tiles.append(x_t)

            # Interleave chunk-pair and x-tile DMAs in consumption order.
            # The very first matmul needs only wc(0,0) + x0, so issue those
            # two first.
            load_chunk(0, 0)
            load_x(0)
            load_chunk(1, 0)
            for kt in range(1, 6):
                load_x(kt)
            # Bias is tiny but descriptor-heavy; keep it off the critical
            # startup path (first needed at the first eviction, ~60us in).
            b_sb = bpool.tile([P, OT], mybir.dt.float32, name="b_sb")
            nc.sync.dma_start(b_sb[:], bv.ap())
            load_chunk_pair(1)
            for kt in range(6, 14):
                load_x(kt)
            load_chunk_pair(2)
            for kt in range(14, 22):
                load_x(kt)
            load_chunk_pair(3)
            for kt in range(22, KT):
                load_x(kt)

            psums1 = [
                [
                    pspool.tile([P, TN], mybir.dt.float32, name=f"ps_{o2}_{tt}", tag="ps")
                    for tt in range(TT)
                ]
                for o2 in range(2)
            ]
            for kt in range(KT):
                for o2 in range(2):
                    lhsT = wc[(o2, kt // CHUNK)][:, kt % CHUNK, :]
                    for tt in range(TT):
                        nc.tensor.matmul(
                            psums1[o2][tt][:],
                            lhsT,
                            x_tiles[kt][:, tt * TN : (tt + 1) * TN],
                            start=(kt == 0),
                            stop=(kt == KT - 1),
                        )
            for o2 in range(2):
                for tt in range(TT):
                    evict(nc, mybir, opool, oT_r, b_sb, psums1[o2][tt], o2, tt)

            # Phase 2: remaining blocks against the resident x.
            for ot in range(2, OT):
                w_sb = wpool.tile(
                    [P, KT, P], mybir.dt.bfloat16, name=f"w_{ot}", tag="w"
                )
                nc.sync.dma_start(w_sb[:], wP.ap()[ot])

                # tt-outer: each PSUM bank finishes its 32-matmul group in a
                # burst and evicts while the next bank accumulates, so
                # evictions never pile up after the block's last matmul.
                for tt in range(TT):
                    psum = pspool.tile(
                        [P, TN], mybir.dt.float32, name=f"ps_{ot}_{tt}", tag="ps"
                    )
                    for kt in range(KT):
                        nc.tensor.matmul(
                            psum[:],
                            w_sb[:, kt, :],
                            x_tiles[kt][:, tt * TN : (tt + 1) * TN],
                            start=(kt == 0),
                            stop=(kt == KT - 1),
                        )
                    evict(nc, mybir, opool, oT_r, b_sb, psum, ot, tt)

    nc.compile()
    return nc


def prepare_inputs(x, weight, bias):
    """Host-side layout prep: transpose + cast per-core shards."""
    bf16 = ml_dtypes.bfloat16
    x = np.asarray(x, dtype=np.float32)
    weight = np.asarray(weight, dtype=np.float32)
    bias = np.asarray(bias, dtype=np.float32)
    w_bin = np.where(weight >= 0, np.float32(1.0), np.float32(-1.0))
    # wP[ot, p, kt, o] = sign(W)[ot*128+o, kt*128+p] — per-ot weight blocks,
    # contiguous along (kt, o) so block DMAs are 8KB-contiguous per partition.
    wP_np = np.ascontiguousarray(
        w_bin.reshape(OT, P, KT, P).transpose(0, 3, 2, 1)
    ).astype(bf16)
    bv_np = np.ascontiguousarray(
        np.asarray(bias, dtype=np.float32).reshape(OT, P).T
    )  # [P, OT]; bias[o] at [o % 128, o // 128]
    in_maps = []
    for b in range(B):
        xT_np = np.ascontiguousarray(x[b].T).astype(bf16)  # [in, tokens]
        in_maps.append({"xT": xT_np, "wP": wP_np, "biasv": bv_np})
    return in_maps


def _ensure_ntff_hook_shim():
    """bass_utils' trace path imports antenv.axon_hooks, which some images
    lack; provide a working shim (or a None hook) so tracing never crashes."""
    import sys
    import types

    try:
        import antenv.axon_hooks  # noqa: F401

        return
    except ImportError:
        pass
    hook = None
    try:
        from trn_agent_boot.trn_boot import _ntff_profile_via_ctypes

        hook = _ntff_profile_via_ctypes("/opt/axon/libaxon_pjrt.so")
    except Exception:
        pass
    mod = types.ModuleType("antenv.axon_hooks")
    mod.get_axon_ntff_profile_hook = lambda: hook
    mod.set_axon_ntff_profile_hook = lambda h: None
    sys.modules["antenv.axon_hooks"] = mod
    try:
        import antenv

        antenv.axon_hooks = mod
    except ImportError:
        pass


def run(in_maps, trace=False, **kwargs):
    global _compiled_nc
    if _compiled_nc is None:
        _compiled_nc = build_program()
    _ensure_ntff_hook_shim()
    from concourse.bass_utils import run_bass_kernel_spmd

    return run_bass_kernel_spmd(
        _compiled_nc, in_maps, list(range(N_CORES)), trace=trace, **kwargs
    )


def kernel(x, weight, bias):
    res = run(prepare_inputs(x, weight, bias))
    out = np.empty((B, T, OUT_F), dtype=np.float32)
    for b in range(B):
        out[b] = res.results[b]["outT"].T
    return out
